# revision 16
# baseline (speedup 1.0000x reference)
"""GNN message passing (gather + weighted segment-sum) on 8 Trainium2 cores.

out[n, :] = sum_{e : dst[e] == n} weight[e] * queue[src[e], :]

Strategy
--------
Edges are sharded by destination window (128 destination nodes per window,
49 windows per core).  Each core:
  * gathers queue[src] rows straight from HBM with `dma_gather`, one
    gather instruction per (chunk of 7 windows, src parity) to amortize
    the Q7 descriptor-generation fixed cost (indices are int16, so the
    50000-row queue is addressed as two parity-interleaved 25000-row
    strided views: even rows / odd rows),
  * builds a weighted one-hot H[e, j] = weight[e] * (dstoff[e] == j)
    per window with two broadcast tensor_tensor ops on the DVE
    (iota compare, then weight multiply),
  * accumulates H.T @ G_hi into a [128, 64] PSUM tile per window on the
    TensorEngine (bf16 x bf16 -> fp32),
  * copies each finished window to SBUF and DMAs it to its slice of the
    output.

Numerics: queue and weight are rounded to bf16 (hi parts only); the
resulting max-norm relative error is ~2.5e-3.

All data-dependent structure (edges per window, padded uniformly across
cores so one SPMD NEFF serves all 8 cores) is computed on the host from the
actual inputs at call time.
"""

import contextlib
import sys

sys.path.insert(0, "/opt/trn_rl_repo")

import ml_dtypes
import numpy as np

import concourse.bass as bass  # noqa: F401
import concourse.mybir as mybir
import concourse.tile as tile
from concourse import bacc
from concourse.bass_utils import run_bass_kernel_spmd

P = 128
N_CORES = 8

N_NODES = 50000
N_EDGES = 800000
D_FEAT = 64


def _plan(n_nodes, n_cores):
    """Windows-per-core and chunking. All cores run the identical program."""
    n_windows = -(-n_nodes // P)
    wpc = -(-n_windows // n_cores)
    # chunk width: largest divisor of wpc that keeps gather tiles a sane size
    cw = max(d for d in range(1, min(wpc, 8) + 1) if wpc % d == 0)
    nchunk = wpc // cw
    return wpc, cw, nchunk


def _host_prep(weight, src, dst, n_nodes, wpc, cw, nchunk, n_cores):
    """Bucket edges by (core, window, src parity); pad uniformly.

    Returns (epw, idx_hbm, aux_hbm, cnt_hbm):
      idx_hbm [n_cores, nchunk, 2, 128, cw*epw//16] int16  (dma_gather layout)
      aux_hbm [n_cores, nchunk, 128, 4*cw*nb] bf16  (dstoff then weight, packed
              so block k of half h of window j sits at column (j*2+h)*nb+k)
      cnt_hbm per-gather valid-edge counts
    """
    e = src.shape[0]
    src = np.asarray(src).astype(np.int64).reshape(-1)
    dst = np.asarray(dst).astype(np.int64).reshape(-1)
    wgt = np.asarray(weight, dtype=np.float32).reshape(-1)

    w = dst >> 7
    core = w // wpc
    lw = w - core * wpc
    half = src & 1
    hidx = (src >> 1).astype(np.int16)
    dstoff = (dst & 127).astype(np.float32)

    nbuckets = n_cores * wpc * 2
    key = (core * wpc + lw) * 2 + half
    # secondary sort key: src, for HBM locality within each gather
    order = np.lexsort((src, key))
    counts = np.bincount(key, minlength=nbuckets)
    epw = int(-(-max(int(counts.max()), 1) // P) * P)
    offs = np.zeros(nbuckets + 1, np.int64)
    np.cumsum(counts, out=offs[1:])
    skey = key[order]
    rank = np.arange(e, dtype=np.int64) - offs[skey]
    dest = skey * epw + rank

    bf = ml_dtypes.bfloat16
    # pads are trailing -1 indices: the gather ucode trims them (no packets)
    idx_arr = np.full(nbuckets * epw, -1, np.int16)
    dst_arr = np.zeros(nbuckets * epw, bf)
    w_arr = np.zeros(nbuckets * epw, bf)
    idx_arr[dest] = hidx[order]
    dst_arr[dest] = dstoff[order].astype(bf)  # 0..127, exact in bf16
    w_arr[dest] = wgt[order].astype(bf)

    nb = epw // P
    big = cw * epw  # indices per chunk-half
    shp = (n_cores, nchunk, cw, 2, epw)
    idx_arr = idx_arr.reshape(shp)
    dst_arr = dst_arr.reshape(shp)
    w_arr = w_arr.reshape(shp)

    # one gather per (chunk, half): windows j < cw-1 keep interior pads as
    # idx 0 (safe row; the one-hot's zero weight kills the contribution),
    # only the final window's tail stays -1 so the ucode can trim it.
    idx_arr[:, :, : cw - 1][idx_arr[:, :, : cw - 1] == -1] = 0

    # idx: window-major edge list per (core, chunk, half), wrapped mod 16 and
    # replicated to 128 partitions (8 Q7 cores each read a 16-partition copy).
    a = idx_arr.transpose(0, 1, 3, 2, 4).reshape(n_cores, nchunk, 2, big // 16, 16)
    a = a.transpose(0, 1, 2, 4, 3)  # [.., 16, big//16]
    idx_hbm = np.broadcast_to(
        a[:, :, :, None, :, :], (n_cores, nchunk, 2, 8, 16, big // 16)
    ).reshape(n_cores, nchunk, 2, P, big // 16)
    idx_hbm = np.ascontiguousarray(idx_hbm)

    def pack(x):
        # window-major block columns: col = (j*2 + h)*nb + k
        y = x.reshape(n_cores, nchunk, cw, 2, nb, P)
        y = y.transpose(0, 1, 5, 2, 3, 4)  # [core, chunk, P, j, h, k]
        return y.reshape(n_cores, nchunk, P, 2 * cw * nb)

    aux_hbm = np.concatenate([pack(dst_arr), pack(w_arr)], axis=3)
    aux_hbm = np.ascontiguousarray(aux_hbm)
    # per-gather valid-edge counts, ordered (chunk, half): all windows but the
    # last are fully emitted (interior pads), the last one trims its tail
    cnt_hbm = np.ascontiguousarray(
        ((cw - 1) * epw + counts.reshape(n_cores, nchunk, cw, 2)[:, :, cw - 1, :])
        .reshape(n_cores, 1, nchunk * 2)
        .astype(np.int32)
    )
    return epw, idx_hbm, aux_hbm, cnt_hbm


ALL_PARTS = frozenset({"gather", "dve", "mm", "out"})


def _build(n_nodes, d, epw, wpc, cw, nchunk, iters=1, parts=ALL_PARTS):
    f32 = mybir.dt.float32
    bf16 = mybir.dt.bfloat16
    nb = epw // P
    big = cw * epw
    bpc = cw * nb  # blocks per half per chunk
    ne = n_nodes // 2
    assert n_nodes % 2 == 0

    nc = bacc.Bacc(
        "TRN2", target_bir_lowering=False, debug=False, num_swdge_queues=4
    )

    # qhl[p] = 256 bf16: [hi(node 2p) | lo(node 2p) | hi(node 2p+1) | lo(node 2p+1)]
    qhl_t = nc.dram_tensor("qhl", [ne, 4 * d], bf16, kind="ExternalInput")
    idx_t = nc.dram_tensor(
        "idx", [nchunk, 2, P, big // 16], mybir.dt.int16, kind="ExternalInput"
    )
    aux_t = nc.dram_tensor("aux", [nchunk, P, 4 * bpc], bf16, kind="ExternalInput")
    iota_t = nc.dram_tensor("iota", [P, P], bf16, kind="ExternalInput")
    cnt_t = nc.dram_tensor(
        "cnt", [1, nchunk * 2], mybir.dt.int32, kind="ExternalInput"
    )
    out_t = nc.dram_tensor("out", [wpc * P, d], f32, kind="ExternalOutput")

    q2 = qhl_t.ap()  # [ne, 4d]
    qviews = [q2[:, 0 : 2 * d], q2[:, 2 * d : 4 * d]]

    with tile.TileContext(nc) as tc:
        gbufs = 2
        with (
            tc.tile_pool(name="const", bufs=1) as cpool,
            tc.tile_pool(name="io", bufs=2) as iopool,
            tc.tile_pool(name="gat", bufs=gbufs) as gpool,
            tc.tile_pool(name="hot", bufs=6) as hpool,
            tc.tile_pool(name="ost", bufs=4) as opool,
            tc.tile_pool(name="ps", bufs=4, space="PSUM") as ppool,
        ):
            iota_f = cpool.tile([P, P], bf16)
            nc.sync.dma_start(out=iota_f[:], in_=iota_t.ap()[:, :])
            cnt = cpool.tile([1, nchunk * 2], mybir.dt.int32)
            nc.sync.dma_start(out=cnt[:], in_=cnt_t.ap()[:, :])
            # pre-zero the gather slots: trimmed (padded) tail positions are
            # never written by the gather, and must not contain NaN patterns
            for h in (0, 1):
                for _ in range(gbufs):
                    gz = gpool.tile([P, cw * nb, 2 * d], bf16, tag=f"g{h}")
                    nc.vector.memset(gz[:], 0)

            loop = tc.For_i(0, iters, 1) if iters > 1 else contextlib.nullcontext()
            with loop:
                for c in range(nchunk):
                    gt = []
                    for h in (0, 1):
                        it = iopool.tile(
                            [P, big // 16], mybir.dt.int16, tag=f"idx{h}"
                        )
                        nc.sync.dma_start(out=it[:], in_=idx_t.ap()[c, h])
                        g = gpool.tile([P, cw * nb, 2 * d], bf16, tag=f"g{h}")
                        if "gather" in parts:
                            gidx = c * 2 + h
                            r = nc.alloc_register(mybir.EngineType.Pool)
                            nc.gpsimd.reg_load(r, cnt[0:1, gidx : gidx + 1])
                            nc.gpsimd.dma_gather(
                                out_ap=g[:],
                                in_ap=qviews[h],
                                idxs_ap=it[:],
                                num_idxs=big,
                                num_idxs_reg=r,
                                elem_size=2 * d,
                                elem_step=4 * d,
                                single_packet=False,
                                queue_num=gidx % 4,
                            )
                        elif "seqload" in parts:
                            flat = qhl_t.ap()[0 : P * 64, :].rearrange(
                                "(p c) d -> p (c d)", p=P
                            )
                            nc.sync.dma_start(
                                out=g[:, 0:nb].rearrange("p a b -> p (a b)"),
                                in_=flat[:, 0 : nb * 2 * d],
                            )
                        gt.append(g)
                    aux = iopool.tile([P, 4 * bpc], bf16, tag="aux")
                    nc.sync.dma_start(out=aux[:], in_=aux_t.ap()[c])

                    for j in range(cw):
                        ps = ppool.tile([P, d], f32)
                        nbw = 2 * nb  # blocks in this window (both halves)
                        wcol = j * nbw  # first block column of this window

                        def bcast(ap2d, n_mid, mid_is_data):
                            # [P, X] -> [P, n_mid, P] AP; data dim keeps its
                            # step, the other dim gets step 0
                            pairs = list(ap2d.ap)
                            assert len(pairs) == 2
                            if mid_is_data:
                                newp = [pairs[0], [pairs[1][0], n_mid], [0, P]]
                            else:
                                newp = [pairs[0], [0, n_mid], pairs[1]]
                            return bass.AP(ap2d.tensor, ap2d.offset, newp)

                        if "dve" in parts:
                            h01 = hpool.tile([P, nbw, P], bf16, tag="h01")
                            nc.vector.tensor_tensor(
                                out=h01[:],
                                in0=bcast(iota_f[:], nbw, False),
                                in1=bcast(aux[:, wcol : wcol + nbw], nbw, True),
                                op=mybir.AluOpType.is_equal,
                            )
                            hw = hpool.tile([P, nbw, P], bf16, tag="hw")
                            nc.vector.tensor_tensor(
                                out=hw[:],
                                in0=h01[:],
                                in1=bcast(
                                    aux[:, 2 * bpc + wcol : 2 * bpc + wcol + nbw],
                                    nbw,
                                    True,
                                ),
                                op=mybir.AluOpType.mult,
                            )
                        if "mm" in parts:
                            for h in (0, 1):
                                for k in range(nb):
                                    bi = h * nb + k  # block within window
                                    lhs = hw[:, bi, :] if "dve" in parts else iota_f[:]
                                    nc.tensor.matmul(
                                        ps[:],
                                        lhsT=lhs,
                                        rhs=gt[h][:, j * nb + k, 0:d],
                                        start=bi == 0,
                                        stop=bi == nbw - 1,
                                    )
                        wg = c * cw + j
                        if "out" in parts and "mm" in parts:
                            ot = opool.tile([P, d], f32, tag="ot")
                            nc.scalar.copy(ot[:], ps[:])
                            nc.sync.dma_start(
                                out=out_t.ap()[wg * P : (wg + 1) * P, :], in_=ot[:]
                            )
                        elif "dve" in parts and "mm" not in parts:
                            # variant build: keep the one-hots alive (anti-DCE)
                            nc.vector.tensor_tensor(
                                out=hw[:, 0, :],
                                in0=hw[:, 0, :],
                                in1=h01[:, 0, :],
                                op=mybir.AluOpType.max,
                            )
                            nc.sync.dma_start(
                                out=out_t.ap()[wg * P : (wg + 1) * P, :],
                                in_=hw[:, 0, :].bitcast(f32),
                            )
                        elif "dve" not in parts and "mm" not in parts:
                            # variant build: keep the loads alive (anti-DCE)
                            nc.sync.dma_start(
                                out=out_t.ap()[wg * P : (wg + 1) * P, :],
                                in_=gt[0][:, j * nb, :].bitcast(f32),
                            )
                            nc.sync.dma_start(
                                out=out_t.ap()[wg * P : (wg + 1) * P, :],
                                in_=gt[1][:, j * nb, :].bitcast(f32),
                            )
    nc.compile()
    return nc


def _make_inputs(queue, idx_hbm, aux_hbm, cnt_hbm, n_cores):
    bf = ml_dtypes.bfloat16
    q = np.asarray(queue, dtype=np.float32)
    hi = q.astype(bf)
    lo = (q - hi.astype(np.float32)).astype(bf)
    ne, d = q.shape[0] // 2, q.shape[1]
    qhl = np.empty((ne, 4 * d), bf)
    qhl[:, 0:d] = hi[0::2]
    qhl[:, d : 2 * d] = lo[0::2]
    qhl[:, 2 * d : 3 * d] = hi[1::2]
    qhl[:, 3 * d : 4 * d] = lo[1::2]
    iota_np = np.ascontiguousarray(
        np.broadcast_to(np.arange(P, dtype=np.float32), (P, P)).astype(bf)
    )
    return [
        {
            "qhl": qhl,
            "idx": idx_hbm[c],
            "aux": aux_hbm[c],
            "iota": iota_np,
            "cnt": cnt_hbm[c],
        }
        for c in range(n_cores)
    ]


def _run(queue, weight, src, dst, n_nodes, d, n_cores, trace=False, iters=1):
    queue = np.ascontiguousarray(np.asarray(queue, dtype=np.float32))
    wpc, cw, nchunk = _plan(n_nodes, n_cores)
    epw, idx_hbm, aux_hbm, cnt_hbm = _host_prep(
        weight, src, dst, n_nodes, wpc, cw, nchunk, n_cores
    )
    nc = _build(n_nodes, d, epw, wpc, cw, nchunk, iters=iters)
    in_maps = _make_inputs(queue, idx_hbm, aux_hbm, cnt_hbm, n_cores)
    res = run_bass_kernel_spmd(nc, in_maps, core_ids=list(range(n_cores)), trace=trace)
    full = np.concatenate([res.results[c]["out"] for c in range(n_cores)], axis=0)
    return full[:n_nodes], res


def kernel(queue, weight, src, dst):
    out, _ = _run(queue, weight, src, dst, N_NODES, D_FEAT, N_CORES)
    return out


# revision 20
# speedup vs baseline: 1.5748x; 1.5748x over previous
"""GNN message passing (gather + weighted segment-sum) on 8 Trainium2 cores.

out[n, :] = sum_{e : dst[e] == n} weight[e] * queue[src[e], :]

Strategy
--------
Edges are sharded by destination window (128 destination nodes per window,
49 windows per core).  Each core:
  * gathers queue[src] rows straight from HBM with `dma_gather`, one
    gather per (window, src parity); trailing -1 pad indices are trimmed
    by the gather ucode so only real edges cost descriptors (indices are
    int16, so the 50000-row queue is addressed as two parity-interleaved
    25000-row strided views: even rows / odd rows),
  * builds a weighted one-hot H[e, j] = weight[e] * (dstoff[e] == j)
    per window with two broadcast tensor_tensor ops on the DVE
    (iota compare, then weight multiply),
  * accumulates H.T @ G_hi into a [128, 64] PSUM tile per window on the
    TensorEngine (bf16 x bf16 -> fp32),
  * copies each finished window to SBUF and DMAs it to its slice of the
    output.

Numerics: queue and weight are rounded to bf16 (hi parts only); the
resulting max-norm relative error is ~2.5e-3.

Descriptor generation on the Q7 cores is the wall: it is serial across
gather instructions (~2.8 ns per emitted index), so the kernel is
structured to keep every other engine's work underneath it.

All data-dependent structure (edges per window, padded uniformly across
cores so one SPMD NEFF serves all 8 cores) is computed on the host from the
actual inputs at call time.
"""

import contextlib
import sys

sys.path.insert(0, "/opt/trn_rl_repo")

import ml_dtypes
import numpy as np

import concourse.bass as bass  # noqa: F401
import concourse.mybir as mybir
import concourse.tile as tile
from concourse import bacc
from concourse.bass_utils import run_bass_kernel_spmd

P = 128
N_CORES = 8

N_NODES = 50000
N_EDGES = 800000
D_FEAT = 64


def _plan(n_nodes, n_cores):
    """Windows-per-core and chunking. All cores run the identical program."""
    n_windows = -(-n_nodes // P)
    wpc = -(-n_windows // n_cores)
    # chunk width: largest divisor of wpc that keeps gather tiles a sane size
    cw = max(d for d in range(1, min(wpc, 8) + 1) if wpc % d == 0)
    nchunk = wpc // cw
    return wpc, cw, nchunk


def _host_prep(weight, src, dst, n_nodes, wpc, cw, nchunk, n_cores):
    """Bucket edges by (core, window, src parity); pad uniformly.

    Returns (epw, idx_hbm, aux_hbm, cnt_hbm):
      idx_hbm [n_cores, nchunk, 2, 128, cw*epw//16] int16  (dma_gather layout)
      aux_hbm [n_cores, nchunk, 128, 4*cw*nb] bf16  (dstoff then weight, packed
              so block k of half h of window j sits at column (j*2+h)*nb+k)
      cnt_hbm per-gather valid-edge counts
    """
    e = src.shape[0]
    src = np.asarray(src).astype(np.int64).reshape(-1)
    dst = np.asarray(dst).astype(np.int64).reshape(-1)
    wgt = np.asarray(weight, dtype=np.float32).reshape(-1)

    w = dst >> 7
    core = w // wpc
    lw = w - core * wpc
    half = src & 1
    hidx = (src >> 1).astype(np.int16)
    dstoff = (dst & 127).astype(np.float32)

    nbuckets = n_cores * wpc * 2
    key = (core * wpc + lw) * 2 + half
    # secondary sort key: src, for HBM locality within each gather
    order = np.lexsort((src, key))
    counts = np.bincount(key, minlength=nbuckets)
    epw = int(-(-max(int(counts.max()), 1) // P) * P)
    offs = np.zeros(nbuckets + 1, np.int64)
    np.cumsum(counts, out=offs[1:])
    skey = key[order]
    rank = np.arange(e, dtype=np.int64) - offs[skey]
    dest = skey * epw + rank

    bf = ml_dtypes.bfloat16
    # pads are trailing -1 indices: the gather ucode trims them (no packets)
    idx_arr = np.full(nbuckets * epw, -1, np.int16)
    dst_arr = np.zeros(nbuckets * epw, bf)
    w_arr = np.zeros(nbuckets * epw, bf)
    idx_arr[dest] = hidx[order]
    dst_arr[dest] = dstoff[order].astype(bf)  # 0..127, exact in bf16
    w_arr[dest] = wgt[order].astype(bf)

    nb = epw // P
    big = cw * epw  # indices per chunk-half
    shp = (n_cores, nchunk, cw, 2, epw)
    idx_arr = idx_arr.reshape(shp)
    dst_arr = dst_arr.reshape(shp)
    w_arr = w_arr.reshape(shp)

    # idx: window-major edge list per (core, chunk, half), wrapped mod 16 and
    # replicated to 128 partitions (8 Q7 cores each read a 16-partition copy).
    a = idx_arr.transpose(0, 1, 3, 2, 4).reshape(n_cores, nchunk, 2, big // 16, 16)
    a = a.transpose(0, 1, 2, 4, 3)  # [.., 16, big//16]
    idx_hbm = np.broadcast_to(
        a[:, :, :, None, :, :], (n_cores, nchunk, 2, 8, 16, big // 16)
    ).reshape(n_cores, nchunk, 2, P, big // 16)
    idx_hbm = np.ascontiguousarray(idx_hbm)

    def pack(x):
        # window-major block columns: col = (j*2 + h)*nb + k
        y = x.reshape(n_cores, nchunk, cw, 2, nb, P)
        y = y.transpose(0, 1, 5, 2, 3, 4)  # [core, chunk, P, j, h, k]
        return y.reshape(n_cores, nchunk, P, 2 * cw * nb)

    aux_hbm = np.concatenate([pack(dst_arr), pack(w_arr)], axis=3)
    aux_hbm = np.ascontiguousarray(aux_hbm)
    # per-gather valid-edge counts, ordered (chunk, window, half)
    cnt_hbm = np.ascontiguousarray(
        counts.reshape(n_cores, nchunk, cw, 2)
        .reshape(n_cores, 1, nchunk * cw * 2)
        .astype(np.int32)
    )
    return epw, idx_hbm, aux_hbm, cnt_hbm


ALL_PARTS = frozenset({"gather", "dve", "mm", "out"})


def _build(n_nodes, d, epw, wpc, cw, nchunk, iters=1, parts=ALL_PARTS):
    f32 = mybir.dt.float32
    bf16 = mybir.dt.bfloat16
    nb = epw // P
    big = cw * epw
    bpc = cw * nb  # blocks per half per chunk
    ne = n_nodes // 2
    assert n_nodes % 2 == 0

    nc = bacc.Bacc(
        "TRN2", target_bir_lowering=False, debug=False, num_swdge_queues=4
    )

    # qhl[p] = 256 bf16: [hi(node 2p) | lo(node 2p) | hi(node 2p+1) | lo(node 2p+1)]
    qhl_t = nc.dram_tensor("qhl", [ne, 4 * d], bf16, kind="ExternalInput")
    idx_t = nc.dram_tensor(
        "idx", [nchunk, 2, P, big // 16], mybir.dt.int16, kind="ExternalInput"
    )
    aux_t = nc.dram_tensor("aux", [nchunk, P, 4 * bpc], bf16, kind="ExternalInput")
    iota_t = nc.dram_tensor("iota", [P, P], bf16, kind="ExternalInput")
    cnt_t = nc.dram_tensor(
        "cnt", [1, nchunk * cw * 2], mybir.dt.int32, kind="ExternalInput"
    )
    out_t = nc.dram_tensor("out", [wpc * P, d], f32, kind="ExternalOutput")

    q2 = qhl_t.ap()  # [ne, 4d]
    qviews = [q2[:, 0 : 2 * d], q2[:, 2 * d : 4 * d]]

    with tile.TileContext(nc) as tc:
        gbufs = 6
        with (
            tc.tile_pool(name="const", bufs=1) as cpool,
            tc.tile_pool(name="io", bufs=2) as iopool,
            tc.tile_pool(name="gat", bufs=gbufs) as gpool,
            tc.tile_pool(name="hot", bufs=6) as hpool,
            tc.tile_pool(name="ost", bufs=4) as opool,
            tc.tile_pool(name="ps", bufs=4, space="PSUM") as ppool,
        ):
            iota_f = cpool.tile([P, P], bf16)
            nc.sync.dma_start(out=iota_f[:], in_=iota_t.ap()[:, :])
            cnt = cpool.tile([1, nchunk * cw * 2], mybir.dt.int32)
            nc.sync.dma_start(out=cnt[:], in_=cnt_t.ap()[:, :])
            # pre-zero the gather slots: trimmed (padded) tail positions are
            # never written by the gather, and must not contain NaN patterns
            for h in (0, 1):
                for _ in range(gbufs):
                    gz = gpool.tile([P, nb, 2 * d], bf16, tag=f"g{h}")
                    nc.vector.memset(gz[:], 0)

            loop = tc.For_i(0, iters, 1) if iters > 1 else contextlib.nullcontext()
            with loop:
                for c in range(nchunk):
                    idxs = []
                    for h in (0, 1):
                        it = iopool.tile(
                            [P, big // 16], mybir.dt.int16, tag=f"idx{h}"
                        )
                        nc.sync.dma_start(out=it[:], in_=idx_t.ap()[c, h])
                        idxs.append(it)
                    aux = iopool.tile([P, 4 * bpc], bf16, tag="aux")
                    nc.sync.dma_start(out=aux[:], in_=aux_t.ap()[c])

                    for j in range(cw):
                        gt = []
                        for h in (0, 1):
                            g = gpool.tile([P, nb, 2 * d], bf16, tag=f"g{h}")
                            if "gather" in parts:
                                sl = epw // 16
                                gidx = (c * cw + j) * 2 + h
                                nreg = nc.alloc_register(mybir.EngineType.Pool)
                                nc.gpsimd.reg_load(
                                    nreg, cnt[0:1, gidx : gidx + 1]
                                )
                                nc.gpsimd.dma_gather(
                                    out_ap=g[:],
                                    in_ap=qviews[h],
                                    idxs_ap=idxs[h][:, j * sl : (j + 1) * sl],
                                    num_idxs=epw,
                                    num_idxs_reg=nreg,
                                    elem_size=2 * d,
                                    elem_step=4 * d,
                                    single_packet=False,
                                    queue_num=gidx % 4,
                                )
                            elif "seqload" in parts:
                                flat = qhl_t.ap()[0 : P * 64, :].rearrange(
                                    "(p c) d -> p (c d)", p=P
                                )
                                nc.sync.dma_start(
                                    out=g[:].rearrange("p a b -> p (a b)"),
                                    in_=flat[:, 0 : nb * 2 * d],
                                )
                            gt.append(g)

                        ps = ppool.tile([P, d], f32)
                        nbw = 2 * nb  # blocks in this window (both halves)
                        wcol = j * nbw  # first block column of this window

                        def bcast(ap2d, n_mid, mid_is_data):
                            # [P, X] -> [P, n_mid, P] AP; data dim keeps its
                            # step, the other dim gets step 0
                            pairs = list(ap2d.ap)
                            assert len(pairs) == 2
                            if mid_is_data:
                                newp = [pairs[0], [pairs[1][0], n_mid], [0, P]]
                            else:
                                newp = [pairs[0], [0, n_mid], pairs[1]]
                            return bass.AP(ap2d.tensor, ap2d.offset, newp)

                        if "dve" in parts:
                            h01 = hpool.tile([P, nbw, P], bf16, tag="h01")
                            nc.vector.tensor_tensor(
                                out=h01[:],
                                in0=bcast(iota_f[:], nbw, False),
                                in1=bcast(aux[:, wcol : wcol + nbw], nbw, True),
                                op=mybir.AluOpType.is_equal,
                            )
                            hw = hpool.tile([P, nbw, P], bf16, tag="hw")
                            nc.vector.tensor_tensor(
                                out=hw[:],
                                in0=h01[:],
                                in1=bcast(
                                    aux[:, 2 * bpc + wcol : 2 * bpc + wcol + nbw],
                                    nbw,
                                    True,
                                ),
                                op=mybir.AluOpType.mult,
                            )
                        if "mm" in parts:
                            for h in (0, 1):
                                for k in range(nb):
                                    bi = h * nb + k  # block within window
                                    lhs = hw[:, bi, :] if "dve" in parts else iota_f[:]
                                    nc.tensor.matmul(
                                        ps[:],
                                        lhsT=lhs,
                                        rhs=gt[h][:, k, 0:d],
                                        start=bi == 0,
                                        stop=bi == nbw - 1,
                                    )
                        wg = c * cw + j
                        if "out" in parts and "mm" in parts:
                            ot = opool.tile([P, d], f32, tag="ot")
                            nc.scalar.copy(ot[:], ps[:])
                            nc.sync.dma_start(
                                out=out_t.ap()[wg * P : (wg + 1) * P, :], in_=ot[:]
                            )
                        elif "dve" in parts and "mm" not in parts:
                            # variant build: keep the one-hots alive (anti-DCE)
                            nc.vector.tensor_tensor(
                                out=hw[:, 0, :],
                                in0=hw[:, 0, :],
                                in1=h01[:, 0, :],
                                op=mybir.AluOpType.max,
                            )
                            nc.sync.dma_start(
                                out=out_t.ap()[wg * P : (wg + 1) * P, :],
                                in_=hw[:, 0, :].bitcast(f32),
                            )
                        elif "dve" not in parts and "mm" not in parts:
                            # variant build: keep the loads alive (anti-DCE)
                            nc.sync.dma_start(
                                out=out_t.ap()[wg * P : (wg + 1) * P, :],
                                in_=gt[0][:, 0, :].bitcast(f32),
                            )
                            nc.sync.dma_start(
                                out=out_t.ap()[wg * P : (wg + 1) * P, :],
                                in_=gt[1][:, 0, :].bitcast(f32),
                            )
    nc.compile()
    return nc


def _make_inputs(queue, idx_hbm, aux_hbm, cnt_hbm, n_cores):
    bf = ml_dtypes.bfloat16
    q = np.asarray(queue, dtype=np.float32)
    hi = q.astype(bf)
    lo = (q - hi.astype(np.float32)).astype(bf)
    ne, d = q.shape[0] // 2, q.shape[1]
    qhl = np.empty((ne, 4 * d), bf)
    qhl[:, 0:d] = hi[0::2]
    qhl[:, d : 2 * d] = lo[0::2]
    qhl[:, 2 * d : 3 * d] = hi[1::2]
    qhl[:, 3 * d : 4 * d] = lo[1::2]
    iota_np = np.ascontiguousarray(
        np.broadcast_to(np.arange(P, dtype=np.float32), (P, P)).astype(bf)
    )
    return [
        {
            "qhl": qhl,
            "idx": idx_hbm[c],
            "aux": aux_hbm[c],
            "iota": iota_np,
            "cnt": cnt_hbm[c],
        }
        for c in range(n_cores)
    ]


def _run(queue, weight, src, dst, n_nodes, d, n_cores, trace=False, iters=1):
    queue = np.ascontiguousarray(np.asarray(queue, dtype=np.float32))
    wpc, cw, nchunk = _plan(n_nodes, n_cores)
    epw, idx_hbm, aux_hbm, cnt_hbm = _host_prep(
        weight, src, dst, n_nodes, wpc, cw, nchunk, n_cores
    )
    nc = _build(n_nodes, d, epw, wpc, cw, nchunk, iters=iters)
    in_maps = _make_inputs(queue, idx_hbm, aux_hbm, cnt_hbm, n_cores)
    res = run_bass_kernel_spmd(nc, in_maps, core_ids=list(range(n_cores)), trace=trace)
    full = np.concatenate([res.results[c]["out"] for c in range(n_cores)], axis=0)
    return full[:n_nodes], res


def kernel(queue, weight, src, dst):
    out, _ = _run(queue, weight, src, dst, N_NODES, D_FEAT, N_CORES)
    return out
